# revision 1
# baseline (speedup 1.0000x reference)
"""MoE expert-parallel kernel for trn2 (8 cores).

Per core r: expert r + home token block [256r, 256(r+1)).
fp32 router (replicated, exact top-2) -> per-block prefix-sum slots ->
indirect-DMA scatter of token ids into per-block slot lists -> indirect
gather of selected bf16 rows -> bf16 expert MLP -> one bf16 AllToAll ->
home-side weighted one-hot combine matmul.

All rank-dependent selection is data-driven (one-hot selector inputs),
so the single SPMD graph is identical across cores.
"""

import numpy as np
import ml_dtypes
import concourse.bass as bass
import concourse.bacc as bacc
import concourse.mybir as mybir
import concourse.tile as tile

P = 128
T = 2048
TB = 16  # token tiles
H = 768
HC = 6  # h chunks
E = 8
I2 = 1536
C2 = 96  # capacity per (home block, expert)
S = E * C2  # 768 expert slots
NCORES = 8
BIG = 1.0e6

f32 = mybir.dt.float32
f32r = mybir.dt.float32r
bf16 = mybir.dt.bfloat16
i32 = mybir.dt.int32
AF = mybir.ActivationFunctionType
ALU = mybir.AluOpType

GLU_MODE = "silu"  # "silu" (hw) or "sigmoid" (simulator-compatible)


def build(glu_mode=GLU_MODE):
    nc = bacc.Bacc("TRN2", target_bir_lowering=False, debug=False)

    xt = nc.declare_dram_parameter("xt", [24, P, 512], f32r, isOutput=False)
    xbf = nc.declare_dram_parameter("xbf", [T, H], bf16, isOutput=False)
    wr = nc.declare_dram_parameter("wr", [H, E], f32r, isOutput=False)
    wgu = nc.declare_dram_parameter("wgu", [H, I2], bf16, isOutput=False)
    wd = nc.declare_dram_parameter("wd", [H, H], bf16, isOutput=False)
    bg = nc.declare_dram_parameter("bg", [P, HC], f32, isOutput=False)
    bga = nc.declare_dram_parameter("bga", [P, HC], f32, isOutput=False)
    bu = nc.declare_dram_parameter("bu", [P, HC], f32, isOutput=False)
    bd_bc = nc.declare_dram_parameter("bd_bc", [P, H], f32, isOutput=False)
    br_col = nc.declare_dram_parameter("br_col", [E, 1], f32, isOutput=False)
    tri = nc.declare_dram_parameter("tri", [P, P], f32, isOutput=False)
    onesq = nc.declare_dram_parameter("onesq", [P, P], f32, isOutput=False)
    idf = nc.declare_dram_parameter("idf", [P, P], f32, isOutput=False)
    idb = nc.declare_dram_parameter("idb", [P, P], bf16, isOutput=False)
    iota_c = nc.declare_dram_parameter("iota_c", [P, C2], f32, isOutput=False)
    tok_iota = nc.declare_dram_parameter("tok_iota", [P, TB], i32, isOutput=False)
    selr = nc.declare_dram_parameter("selr", [P, E], f32, isOutput=False)
    tsel = nc.declare_dram_parameter("tsel", [P, 2 * TB], f32, isOutput=False)

    out = nc.declare_dram_parameter("out", [2 * P, H], f32, isOutput=True)

    gx = [nc.dram_tensor(f"gx{b}", [C2, H], bf16) for b in range(E)]
    comb_in = nc.dram_tensor("comb_in", [E, C2, H], bf16)
    comb_out = nc.dram_tensor("comb_out", [E, C2, H], bf16)

    with tile.TileContext(nc) as tc:
        with (
            tc.tile_pool(name="cst", bufs=1) as cst,
            tc.tile_pool(name="rt8", bufs=8) as rt8,
            tc.tile_pool(name="rt", bufs=2) as rt,
            tc.tile_pool(name="wp", bufs=1) as wp,
            tc.tile_pool(name="ml1", bufs=1) as ml1,
            tc.tile_pool(name="ml", bufs=3) as ml,
            tc.tile_pool(name="pa", bufs=1, space="PSUM") as pa,
            tc.tile_pool(name="pb", bufs=2, space="PSUM") as pb,
            tc.tile_pool(name="pc", bufs=1, space="PSUM") as pc,
        ):
            # ---------- small constants ----------
            def cdma(shape, dt_, src):
                t_ = cst.tile(shape, dt_)
                nc.sync.dma_start(out=t_[:], in_=src)
                return t_

            idf_sb = cdma([P, P], f32, idf[:])
            wr_sb = cst.tile([P, HC * E], f32r)
            nc.sync.dma_start(
                out=wr_sb[:].rearrange("p (c e) -> p c e", e=E),
                in_=wr.ap().rearrange("(c p) e -> p c e", p=P),
            )
            br_sb = cdma([E, 1], f32, br_col[:])
            tri_sb = cdma([P, P], f32, tri[:])
            ones_sb = cdma([P, P], f32, onesq[:])
            idb_sb = cdma([P, P], bf16, idb[:])
            iota_sb = cdma([P, C2], f32, iota_c[:])
            selr_sb = cdma([P, E], f32, selr[:])
            tsel_sb = cdma([P, 2 * TB], f32, tsel[:])
            bg_sb = cdma([P, HC], f32, bg[:])
            bga_sb = cdma([P, HC], f32, bga[:])
            bu_sb = cdma([P, HC], f32, bu[:])
            bd_sb = cdma([P, H], f32, bd_bc[:])

            # ---------- router matmuls (start ASAP) ----------
            logits_b = rt.tile([P, TB * E], f32, tag="logits")
            lsb_t = []
            for g in range(4):  # groups of 512 tokens
                xr_tiles = []
                for c in range(HC):
                    t_ = rt8.tile([P, 512], f32r, tag="xr")
                    nc.sync.dma_start(out=t_[:], in_=xt[g * HC + c])
                    xr_tiles.append(t_)
                lps = pc.tile([E, 512], f32, tag="lps")
                for c in range(HC):
                    nc.tensor.matmul(
                        lps[:],
                        lhsT=wr_sb[:, c * E : (c + 1) * E],
                        rhs=xr_tiles[c][:],
                        start=(c == 0),
                        stop=(c == HC - 1),
                    )
                lsb = rt.tile([E, 512], f32, tag=f"lsb{g}")
                nc.vector.tensor_scalar(
                    out=lsb[:], in0=lps[:], scalar1=br_sb[:, 0:1], scalar2=None,
                    op0=ALU.add,
                )
                lsb_t.append(lsb)
                for k in range(4):
                    tp = pb.tile([P, P], f32, tag="tsp")
                    nc.tensor.transpose(
                        tp[:, :E], lsb[:, k * P : (k + 1) * P], idf_sb[:E, :E]
                    )
                    ti = g * 4 + k
                    nc.scalar.activation(
                        logits_b[:, ti * E : (ti + 1) * E], tp[:, :E], AF.Copy
                    )

            # xbf tiles for the x-row scatter (sources live in SBUF)
            xbf_sb = []
            for t_ in range(TB):
                xb = ml1.tile([P, H], bf16, tag=f"xbf{t_}")
                nc.sync.dma_start(out=xb[:], in_=xbf[t_ * P : (t_ + 1) * P, :])
                xbf_sb.append(xb)

            # ---------- expert weights (after router issues) ----------
            wgu_sb = []
            wd_sb = []
            for c in range(HC):
                t_ = wp.tile([P, I2], bf16, tag=f"wgu{c}")
                nc.sync.dma_start(out=t_[:], in_=wgu[c * P : (c + 1) * P, :])
                wgu_sb.append(t_)
            for c in range(HC):
                t_ = wp.tile([P, H], bf16, tag=f"wd{c}")
                nc.sync.dma_start(out=t_[:], in_=wd[c * P : (c + 1) * P, :])
                wd_sb.append(t_)

            # ---------- routing epilogue (batched [P, (TB, E)]) ----------
            l3 = logits_b[:].rearrange("p (t e) -> p t e", e=E)
            m1 = rt.tile([P, TB], f32, tag="m1")
            nc.vector.tensor_reduce(
                out=m1[:], in_=l3, axis=mybir.AxisListType.X, op=ALU.max
            )
            m1b = m1[:].rearrange("p (t o) -> p t o", o=1).to_broadcast([P, TB, E])
            ismax = rt.tile([P, TB * E], f32, tag="ismax")
            i3 = ismax[:].rearrange("p (t e) -> p t e", e=E)
            nc.vector.tensor_tensor(out=i3, in0=l3, in1=m1b, op=ALU.is_ge)
            lm = rt.tile([P, TB * E], f32, tag="lm")
            lm3 = lm[:].rearrange("p (t e) -> p t e", e=E)
            nc.vector.tensor_scalar(
                out=lm3, in0=i3, scalar1=-1.0e9, scalar2=None, op0=ALU.mult
            )
            nc.vector.tensor_tensor(out=lm3, in0=lm3, in1=l3, op=ALU.add)
            m2 = rt.tile([P, TB], f32, tag="m2")
            nc.vector.tensor_reduce(
                out=m2[:], in_=lm3, axis=mybir.AxisListType.X, op=ALU.max
            )
            m2b = m2[:].rearrange("p (t o) -> p t o", o=1).to_broadcast([P, TB, E])
            mask = rt.tile([P, TB * E], f32, tag="mask")
            k3 = mask[:].rearrange("p (t e) -> p t e", e=E)
            nc.vector.tensor_tensor(out=k3, in0=l3, in1=m2b, op=ALU.is_ge)

            # prefix sums -> pos (1-indexed within (block, expert))
            pp = pc.tile([P, TB * E], f32, tag="pp")
            nc.tensor.matmul(pp[:], lhsT=tri_sb[:], rhs=mask[:], start=True, stop=False)
            for b in range(E):
                nc.tensor.matmul(
                    pp[:, (2 * b + 1) * E : (2 * b + 2) * E],
                    lhsT=ones_sb[:],
                    rhs=mask[:, (2 * b) * E : (2 * b + 1) * E],
                    start=False,
                    stop=(b == E - 1),
                )
            # scatter-critical: slot_all = mask*(pp - BIG) + (BIG - 1)
            slot_all = rt.tile([P, TB * E], f32, tag="slotall")
            nc.vector.tensor_scalar(
                out=slot_all[:], in0=pp[:], scalar1=-BIG, scalar2=None, op0=ALU.add
            )
            nc.vector.tensor_tensor(
                out=slot_all[:], in0=slot_all[:], in1=mask[:], op=ALU.mult
            )
            nc.vector.tensor_scalar(
                out=slot_all[:], in0=slot_all[:], scalar1=BIG - 1.0, scalar2=None,
                op0=ALU.add,
            )
            sa3 = slot_all[:].rearrange("p (t e) -> p t e", e=E)
            selb = selr_sb[:].rearrange("p (o e) -> p o e", o=1).to_broadcast([P, TB, E])
            tmp_te = rt.tile([P, TB * E], f32, tag="tmpte")
            t3 = tmp_te[:].rearrange("p (t e) -> p t e", e=E)
            nc.vector.tensor_tensor(out=t3, in0=sa3, in1=selb, op=ALU.mult)
            joint = rt.tile([P, 2 * TB], i32, tag="joint")
            nc.sync.dma_start(out=joint[:, 0:TB], in_=tok_iota[:])
            with nc.allow_low_precision(reason="exact small ints in i32 reduce"):
                nc.vector.tensor_reduce(
                    out=joint[:, TB : 2 * TB], in_=t3,
                    axis=mybir.AxisListType.X, op=ALU.add,
                )

            # zero-fill dispatch buffers (pads -> zero rows)
            zx = rt.tile([P, H], bf16, tag="zx")
            nc.vector.memset(zx[:], 0)
            for b in range(E):
                nc.sync.dma_start(out=gx[b].ap(), in_=zx[0:C2, :])
            # scatter x rows to expert slots; even tiles first, then odd
            for t_ in [x for x in range(TB) if x % 2 == 0] + [
                x for x in range(TB) if x % 2 == 1
            ]:
                nc.gpsimd.indirect_dma_start(
                    out=gx[t_ // 2].ap(),
                    out_offset=bass.IndirectOffsetOnAxis(
                        ap=joint[:, TB + t_ : TB + t_ + 1], axis=0
                    ),
                    in_=xbf_sb[t_][:],
                    in_offset=None,
                    bounds_check=C2 - 1,
                    oob_is_err=False,
                )
            # posm1 for G build (off scatter-critical path)
            posm1 = rt.tile([P, TB * E], f32, tag="posm1")
            nc.vector.tensor_tensor(out=posm1[:], in0=pp[:], in1=mask[:], op=ALU.mult)
            nc.vector.tensor_scalar(
                out=posm1[:], in0=posm1[:], scalar1=-1.0, scalar2=None, op0=ALU.add
            )
            # ---------- home-role G build (independent of MLP; before a2a) ----
            pm_et = posm1[:].rearrange("p (t e) -> p e t", e=E)
            # w = sigmoid(2l - m1 - m2) masked later by G's is_equal vs posm1
            s12 = rt.tile([P, TB], f32, tag="s12")
            nc.vector.tensor_tensor(out=s12[:], in0=m1[:], in1=m2[:], op=ALU.add)
            s12b = s12[:].rearrange("p (t o) -> p t o", o=1).to_broadcast([P, TB, E])
            arg = rt.tile([P, TB * E], f32, tag="arg")
            a3 = arg[:].rearrange("p (t e) -> p t e", e=E)
            nc.vector.tensor_scalar(
                out=a3, in0=l3, scalar1=2.0, scalar2=None, op0=ALU.mult
            )
            nc.vector.tensor_tensor(out=a3, in0=a3, in1=s12b, op=ALU.subtract)
            wgt = rt.tile([P, TB * E], f32, tag="wgt")
            nc.scalar.activation(wgt[:], arg[:], AF.Sigmoid)
            w_et = wgt[:].rearrange("p (t e) -> p e t", e=E)
            own_pos = []
            own_w = []
            tmp_et = rt.tile([P, E * TB], f32, tag="tmpet")
            e3 = tmp_et[:].rearrange("p (e t) -> p e t", t=TB)
            for j in range(2):
                tselb = (
                    tsel_sb[:, j * TB : (j + 1) * TB]
                    .rearrange("p (o t) -> p o t", o=1)
                    .to_broadcast([P, E, TB])
                )
                op_ = rt.tile([P, E], f32, tag=f"ownp{j}")
                nc.vector.tensor_tensor(out=e3, in0=pm_et, in1=tselb, op=ALU.mult)
                nc.vector.tensor_reduce(
                    out=op_[:], in_=e3, axis=mybir.AxisListType.X, op=ALU.add
                )
                own_pos.append(op_)
                ow_ = rt.tile([P, E], f32, tag=f"ownw{j}")
                nc.vector.tensor_tensor(out=e3, in0=w_et, in1=tselb, op=ALU.mult)
                nc.vector.tensor_reduce(
                    out=ow_[:], in_=e3, axis=mybir.AxisListType.X, op=ALU.add
                )
                own_w.append(ow_)
            G = []
            for e in range(E):
                g_ = ml1.tile([C2, 2 * P], bf16, tag=f"G{e}")
                G.append(g_)
            for e in range(E):
                for j in range(2):
                    pw = ml.tile([P, C2], bf16, tag="pw")
                    nc.vector.tensor_scalar(
                        out=pw[:], in0=iota_sb[:],
                        scalar1=own_pos[j][:, e : e + 1],
                        scalar2=own_w[j][:, e : e + 1],
                        op0=ALU.is_equal, op1=ALU.mult,
                    )
                    gt = pb.tile([C2, P], bf16, tag="tsp")
                    nc.tensor.transpose(gt[:], pw[:], idb_sb[:])
                    nc.scalar.activation(G[e][:, j * P : (j + 1) * P], gt[:], AF.Copy)

            # ---------- gather + transpose to x^T ----------
            xT = []
            for c in range(HC):
                t_ = ml1.tile([P, S], bf16, tag=f"xT{c}")
                xT.append(t_)
            for b in range(E):
                xg1 = ml1.tile([C2, H], bf16, tag=f"xg{b}")
                nc.sync.dma_start(out=xg1[:], in_=gx[b].ap())
                for c in range(HC):
                    tp = pb.tile([P, P], bf16, tag="tsp")
                    nc.tensor.transpose(
                        tp[:, :C2], xg1[:, c * P : (c + 1) * P], idb_sb[:C2, :C2]
                    )
                    nc.scalar.activation(
                        xT[c][:, b * C2 : (b + 1) * C2], tp[:, :C2], AF.Copy
                    )

            # ---------- expert MLP ----------
            actT = []
            for c in range(HC):
                t_ = ml1.tile([P, S], bf16, tag=f"actT{c}")
                actT.append(t_)
            for f in range(HC):  # gate tile f pairs with up tile f+6
                gps = {}
                for half, (h0, hw) in enumerate(((0, 512), (512, 256))):
                    for which, foff in (("g", 0), ("u", HC)):
                        ps = pa.tile([P, 512], f32, tag=f"gu{which}{half}")
                        fi = f + foff
                        for c in range(HC):
                            nc.tensor.matmul(
                                ps[:, 0:hw],
                                lhsT=wgu_sb[c][:, fi * P : (fi + 1) * P],
                                rhs=xT[c][:, h0 : h0 + hw],
                                start=(c == 0),
                                stop=(c == HC - 1),
                            )
                        gps[(which, half)] = ps
                for half, (h0, hw) in enumerate(((0, 512), (512, 256))):
                    gp = gps[("g", half)]
                    up = gps[("u", half)]
                    if glu_mode == "silu":
                        glu = ml.tile([P, 512], f32, tag="glu")
                        nc.scalar.activation(
                            glu[:, 0:hw], gp[:, 0:hw], AF.Silu,
                            bias=bga_sb[:, f : f + 1], scale=1.702,
                        )
                    else:
                        sg = ml.tile([P, 512], f32, tag="sg")
                        nc.scalar.activation(
                            sg[:, 0:hw], gp[:, 0:hw], AF.Sigmoid,
                            bias=bga_sb[:, f : f + 1], scale=1.702,
                        )
                        gb = ml.tile([P, 512], f32, tag="gb")
                        nc.vector.tensor_scalar(
                            out=gb[:, 0:hw], in0=gp[:, 0:hw],
                            scalar1=bg_sb[:, f : f + 1], scalar2=None, op0=ALU.add,
                        )
                        glu = ml.tile([P, 512], f32, tag="glu")
                        nc.vector.tensor_tensor(
                            out=glu[:, 0:hw], in0=gb[:, 0:hw], in1=sg[:, 0:hw],
                            op=ALU.mult,
                        )
                    ub = ml.tile([P, 512], f32, tag="ub")
                    nc.vector.tensor_scalar(
                        out=ub[:, 0:hw], in0=up[:, 0:hw],
                        scalar1=bu_sb[:, f : f + 1], scalar2=None, op0=ALU.add,
                    )
                    nc.vector.tensor_tensor(
                        out=actT[f][:, h0 : h0 + hw], in0=glu[:, 0:hw],
                        in1=ub[:, 0:hw], op=ALU.mult,
                    )

            # down + bias -> comb_in rows
            cin_flat = comb_in.ap().rearrange("e c h -> (e c) h")
            for tt_ in range(HC):
                psa = pa.tile([P, 512], f32, tag="gug0")
                psb = pa.tile([P, 256], f32, tag="guu0")
                for c in range(HC):
                    lhs = actT[c][:, tt_ * P : (tt_ + 1) * P]
                    nc.tensor.matmul(
                        psa[:], lhsT=lhs, rhs=wd_sb[c][:, 0:512],
                        start=(c == 0), stop=(c == HC - 1),
                    )
                for c in range(HC):
                    lhs = actT[c][:, tt_ * P : (tt_ + 1) * P]
                    nc.tensor.matmul(
                        psb[:], lhsT=lhs, rhs=wd_sb[c][:, 512:768],
                        start=(c == 0), stop=(c == HC - 1),
                    )
                dn = ml.tile([P, H], bf16, tag="dn")
                nc.vector.tensor_tensor(
                    out=dn[:, 0:512], in0=psa[:], in1=bd_sb[:, 0:512], op=ALU.add
                )
                nc.vector.tensor_tensor(
                    out=dn[:, 512:768], in0=psb[:], in1=bd_sb[:, 512:768], op=ALU.add
                )
                nc.sync.dma_start(out=cin_flat[tt_ * P : (tt_ + 1) * P, :], in_=dn[:])

            # ---------- all-to-all combine ----------
            nc.gpsimd.collective_compute(
                "AllToAll",
                ALU.bypass,
                replica_groups=[list(range(NCORES))],
                ins=[comb_in.ap().opt()],
                outs=[comb_out.ap().opt()],
            )

            # ---------- weighted combine ----------
            rcv = []
            for e in range(E):
                r_ = ml1.tile([C2, H], bf16, tag=f"rcv{e}")
                nc.sync.dma_start(out=r_[:], in_=comb_out[e, :, :])
                rcv.append(r_)
            for j in range(2):
                poa = pa.tile([P, 512], f32, tag="gug1")
                pob = pa.tile([P, 256], f32, tag="guu1")
                for e in range(E):
                    nc.tensor.matmul(
                        poa[:], lhsT=G[e][:, j * P : (j + 1) * P], rhs=rcv[e][:, 0:512],
                        start=(e == 0), stop=(e == E - 1),
                    )
                for e in range(E):
                    nc.tensor.matmul(
                        pob[:], lhsT=G[e][:, j * P : (j + 1) * P],
                        rhs=rcv[e][:, 512:768],
                        start=(e == 0), stop=(e == E - 1),
                    )
                osb = ml.tile([P, H], f32, tag="osb")
                nc.vector.tensor_copy(out=osb[:, 0:512], in_=poa[:])
                nc.vector.tensor_copy(out=osb[:, 512:768], in_=pob[:])
                nc.sync.dma_start(out=out[j * P : (j + 1) * P, :], in_=osb[:])
    nc.compile()
    return nc


def make_in_maps(inputs, glu_mode=GLU_MODE):
    x = np.ascontiguousarray(
        np.asarray(inputs["hidden_states"], dtype=np.float32).reshape(T, H)
    )
    Wr = np.asarray(inputs["Wr"], dtype=np.float32)
    br = np.asarray(inputs["br"], dtype=np.float32)
    Wgu = np.asarray(inputs["Wgu"], dtype=np.float32)
    bgu = np.asarray(inputs["bgu"], dtype=np.float32)
    Wd = np.asarray(inputs["Wd"], dtype=np.float32)
    bd = np.asarray(inputs["bd"], dtype=np.float32)

    xtv = x.T.reshape(H, 4, 512).transpose(1, 0, 2)  # [g][h][512]
    xtc = np.ascontiguousarray(
        xtv.reshape(4, HC, P, 512).reshape(24, P, 512)
    )
    xbf = x.astype(ml_dtypes.bfloat16)
    tri_ = np.triu(np.ones((P, P), np.float32))
    onesq_ = np.ones((P, P), np.float32)
    idf_ = np.eye(P, dtype=np.float32)
    idb_ = np.eye(P).astype(ml_dtypes.bfloat16)
    iota_c_ = np.tile(np.arange(C2, dtype=np.float32), (P, 1))
    tok_iota_ = (
        np.arange(TB, dtype=np.int32)[None, :] * P
        + np.arange(P, dtype=np.int32)[:, None]
    ).astype(np.int32)
    br_col_ = br.reshape(E, 1)

    in_maps = []
    for r in range(NCORES):
        bg_cols = bgu[r, :H].reshape(HC, P).T.astype(np.float32)
        bu_cols = (bgu[r, H:] + 1.0).reshape(HC, P).T.astype(np.float32)
        bga_cols = 1.702 * bg_cols
        if glu_mode == "silu":
            wd_r = (Wd[r] / 1.702).astype(ml_dtypes.bfloat16)
        else:
            wd_r = Wd[r].astype(ml_dtypes.bfloat16)
        selr_ = np.tile(np.eye(E, dtype=np.float32)[r], (P, 1))
        tsel_ = np.zeros((P, 2 * TB), np.float32)
        tsel_[:, 2 * r] = 1.0
        tsel_[:, TB + 2 * r + 1] = 1.0
        in_maps.append(
            dict(
                xt=xtc, xbf=xbf, wr=Wr,
                wgu=Wgu[r].astype(ml_dtypes.bfloat16), wd=wd_r,
                bg=np.ascontiguousarray(bg_cols),
                bga=np.ascontiguousarray(bga_cols),
                bu=np.ascontiguousarray(bu_cols),
                bd_bc=np.tile(bd[r], (P, 1)),
                br_col=br_col_, tri=tri_, onesq=onesq_, idf=idf_, idb=idb_,
                iota_c=iota_c_, tok_iota=tok_iota_, selr=selr_, tsel=tsel_,
            )
        )
    return in_maps


def assemble(results):
    return np.concatenate([results[r]["out"] for r in range(NCORES)], axis=0).reshape(
        2, 1024, H
    )


LAST_EXEC_NS = None


def kernel(**inputs):
    """Full-input entry point: shards across 8 NeuronCores internally."""
    global LAST_EXEC_NS
    from concourse.bass_utils import run_bass_kernel_spmd

    nc = build()
    in_maps = make_in_maps(inputs)
    res = run_bass_kernel_spmd(nc, in_maps, core_ids=list(range(NCORES)))
    LAST_EXEC_NS = res.exec_time_ns
    out = assemble(res.results)
    return out.astype(np.float32)
